# revision 6
# baseline (speedup 1.0000x reference)
"""Trainium2 Bass kernel for nn_CIFAR10_Monarch_MLP2 (4-layer Monarch MLP + log_softmax).

Strategy
--------
Data-parallel over 8 NeuronCores: each core computes 2048 rows of the
16384-row batch with replicated weights; outputs are concatenated on host.

Per core, activations are kept feature-major ([feature partitions, batch
free]) so the block-diagonal matmuls chain without transposes.  The monarch
permutation (flat index k*q+qq -> plane l=(f%4), row r=f//4) is folded into a
host-side re-arrangement of the weights:

 * w1 rows of block k are regrouped by destination plane l, each group padded
   to a fixed `chunk` (multiple of 32), so mm1's PSUM output tiles are
   "plane-pure": every 128-partition PSUM tile belongs to a single plane and
   is written by 1-4 column-offset matmuls (32-aligned tile_position), one per
   contributing source block.
 * w2 columns are permuted to match the resulting plane-row order (pad rows
   get zero columns), so no data movement is needed for the permutation.

x arrives batch-major in HBM; it is DMA'd contiguously, cast to the matmul
dtype, and transposed on the PE (128x128 identity transposes) into
feature-major tiles.  The final layer's second matmul swaps stationary and
moving operands (activations become lhsT) so its output lands batch-major,
where log_softmax is a cheap free-dim reduction, and results DMA straight out.
"""

import numpy as np
import ml_dtypes

import concourse.bass as bass
from concourse import bacc
import concourse.mybir as mybir
import concourse.tile as tile
from concourse.bass_utils import run_bass_kernel_spmd
from concourse.masks import make_identity

F32 = mybir.dt.float32

# matmul operand dtype knob: mybir.dt.bfloat16 | float32r | float32
MM_DT = mybir.dt.bfloat16

N_CORES = 8
BATCH = 16384
B_CORE = BATCH // N_CORES  # 2048
NB = 256  # batch-tile free size (PSUM bank = 2KB -> 512 fp32 max; 256 = half)

# (p_dim, q_dim, chunk, s_dim) per layer; QPAD = RPAD = 4*chunk
LAYER_CFG = [
    (768, 750, 192, 750),
    (750, 250, 64, 250),
    (250, 25, 32, 25),
    (25, 3, 32, 3),
]


def _np_mmdt():
    return {
        mybir.dt.bfloat16: ml_dtypes.bfloat16,
        mybir.dt.float32r: np.float32,
        mybir.dt.float32: np.float32,
    }[MM_DT]


def arrange_layer(w1, w2, q_dim, chunk):
    """w1:(4,q,p), w2:(4,s,r=q) -> w1t:[4,p,QPAD] (mm1 lhsT), w2t:[4,QPAD,s]
    (mm2 lhsT), with the monarch permutation folded in (see module doc)."""
    nb, _, p_dim = w1.shape
    s_dim = w2.shape[1]
    QPAD = 4 * chunk
    w1t = np.zeros((nb, p_dim, QPAD), np.float32)
    w2t = np.zeros((nb, QPAD, s_dim), np.float32)
    for k in range(nb):
        for l in range(nb):
            qs = [q for q in range(q_dim) if (k * q_dim + q) % 4 == l]
            w1t[k, :, l * chunk : l * chunk + len(qs)] = w1[k, qs, :].T
            rs = [(k * q_dim + q) // 4 for q in qs]
            w2t[l, k * chunk : k * chunk + len(qs), :] = w2[l, :, rs]
    return w1t, w2t


def pieces_for_tile(t, chunk):
    """Pieces of plane-tile t (rows [128t,128t+128)) on the block-chunk grid:
    (psum_part_start, psum_part_end, block_k, within_chunk_row_start)."""
    out = []
    for k in range(4):
        a = max(128 * t, k * chunk)
        b = min(128 * t + 128, (k + 1) * chunk)
        if a < b:
            out.append((a - 128 * t, b - 128 * t, k, a - k * chunk))
    return out


def ktiles(p_dim):
    """[(row0, size), ...] 128-partition contraction tiles covering p_dim."""
    return [(r, min(128, p_dim - r)) for r in range(0, p_dim, 128)]


def prepare_weights(inputs):
    """Host-side arrangement of all weights/biases into DRAM-parameter arrays."""
    npdt = _np_mmdt()
    arrs = {}
    for li, (p_dim, q_dim, chunk, s_dim) in enumerate(LAYER_CFG, 1):
        w1 = np.asarray(inputs[f"w1_{li}"], np.float32)
        w2 = np.asarray(inputs[f"w2_{li}"], np.float32)
        w1t, w2t = arrange_layer(w1, w2, q_dim, chunk)
        arrs[f"w1t_{li}"] = w1t.astype(npdt)
        if li < 4:
            arrs[f"w2t_{li}"] = w2t.astype(npdt)
            bias = np.asarray(inputs[f"b{li}"], np.float32)  # [4*s_dim], f'=l*s+s
            # bias columns per (plane l, s-tile mt): [128, ncols]
            mts = ktiles(s_dim)
            cols = np.zeros((128, 4 * len(mts)), np.float32)
            for l in range(4):
                for mi, (m0, msz) in enumerate(mts):
                    cols[:msz, l * len(mts) + mi] = bias[l * s_dim + m0 : l * s_dim + m0 + msz]
            arrs[f"bias_{li}"] = cols
        else:
            # L4: w2big [4, QPAD, 12]: plane l writes cols [3l,3l+3) of f'=l*3+s
            QPAD = 4 * chunk
            w2big = np.zeros((4, QPAD, 12), np.float32)
            for l in range(4):
                w2big[l, :, 3 * l : 3 * l + 3] = w2t[l]
            arrs["w2big_4"] = w2big.astype(npdt)
            b4 = np.asarray(inputs["b4"], np.float32)
            b4r = np.zeros((1, 12), np.float32)
            b4r[0, :10] = b4
            arrs["b4r"] = b4r.astype(npdt)
    return arrs


def build_nc(b_core=B_CORE):
    """Build the single-core Bass program (SPMD: same program, per-core x)."""
    nc = bacc.Bacc(None, target_bir_lowering=False)
    x_d = nc.declare_dram_parameter("x", [b_core, 3072], F32, isOutput=False)
    y_d = nc.declare_dram_parameter("y", [b_core, 10], F32, isOutput=True)

    wd = {}
    for li, (p_dim, q_dim, chunk, s_dim) in enumerate(LAYER_CFG, 1):
        QPAD = 4 * chunk
        wd[f"w1t_{li}"] = nc.declare_dram_parameter(
            f"w1t_{li}", [4, p_dim, QPAD], MM_DT, isOutput=False)
        if li < 4:
            wd[f"w2t_{li}"] = nc.declare_dram_parameter(
                f"w2t_{li}", [4, QPAD, s_dim], MM_DT, isOutput=False)
            nmt = len(ktiles(s_dim))
            wd[f"bias_{li}"] = nc.declare_dram_parameter(
                f"bias_{li}", [128, 4 * nmt], F32, isOutput=False)
        else:
            wd["w2big_4"] = nc.declare_dram_parameter(
                "w2big_4", [4, QPAD, 12], MM_DT, isOutput=False)
            wd["b4r"] = nc.declare_dram_parameter("b4r", [1, 12], MM_DT, isOutput=False)

    n_bt = b_core // NB

    with tile.TileContext(nc) as tc:
        with (
            tc.tile_pool(name="const", bufs=1) as const,
            tc.tile_pool(name="xload", bufs=3) as xload,
            tc.tile_pool(name="xcast", bufs=3) as xcast,
            tc.tile_pool(name="xT", bufs=2) as xTp,
            tc.tile_pool(name="acts", bufs=1) as acts,
            tc.tile_pool(name="psum_t", bufs=2, space="PSUM") as psum_t,
            tc.tile_pool(name="psum_mm", bufs=4, space="PSUM") as psum_mm,
            tc.tile_pool(name="psum_s", bufs=2, space="PSUM") as psum_s,
            tc.tile_pool(name="sm", bufs=2) as smp,
        ):
            # ---- resident constants ----
            ident = const.tile([128, 128], MM_DT, name="ident", tag="ident")
            make_identity(nc, ident)
            ones_row = const.tile([1, 128], MM_DT, name="ones_row", tag="ones_row")
            nc.any.memset(ones_row[:], 1.0)

            w1sb, w2sb, biassb = {}, {}, {}
            for li, (p_dim, q_dim, chunk, s_dim) in enumerate(LAYER_CFG, 1):
                QPAD = 4 * chunk
                kts = ktiles(p_dim)
                w1sb[li] = const.tile([128, len(kts) * 4 * QPAD], MM_DT, name=f"w1sb{li}", tag=f"w1sb{li}")
                for k in range(4):
                    for ki, (k0, ksz) in enumerate(kts):
                        col = (k * len(kts) + ki) * QPAD
                        nc.sync.dma_start(
                            w1sb[li][:ksz, col : col + QPAD],
                            wd[f"w1t_{li}"][k, k0 : k0 + ksz, :],
                        )
                if li < 4:
                    nrt = QPAD // 128
                    w2sb[li] = const.tile([128, 4 * nrt * s_dim], MM_DT, name=f"w2sb{li}", tag=f"w2sb{li}")
                    for l in range(4):
                        for rt in range(nrt):
                            col = (l * nrt + rt) * s_dim
                            nc.sync.dma_start(
                                w2sb[li][:, col : col + s_dim],
                                wd[f"w2t_{li}"][l, 128 * rt : 128 * (rt + 1), :],
                            )
                    nmt = len(ktiles(s_dim))
                    biassb[li] = const.tile([128, 4 * nmt], F32, name=f"biassb{li}", tag=f"biassb{li}")
                    nc.sync.dma_start(biassb[li][:], wd[f"bias_{li}"][:, :])
                else:
                    w2sb[4] = const.tile([128, 4 * 12], MM_DT, name="w2sb4", tag="w2sb4")
                    for l in range(4):
                        nc.sync.dma_start(
                            w2sb[4][:, l * 12 : (l + 1) * 12],
                            wd["w2big_4"][l, :, :],
                        )
                    biassb[4] = const.tile([1, 12], MM_DT, name="b4rsb", tag="b4rsb")
                    nc.sync.dma_start(biassb[4][:], wd["b4r"][:, :])

            # ---- batch-tile pipeline ----
            for bt in range(n_bt):
                nsub = NB // 128
                # x: load batch-major, cast, PE-transpose to feature-major
                xT = xTp.tile([128, 24 * NB], MM_DT, name="xT", tag="xT")
                for sub in range(nsub):
                    row0 = bt * NB + sub * 128
                    for kc in range(4):  # 768-col chunks
                        xld = xload.tile([128, 768], F32, name="xld", tag="xld")
                        nc.sync.dma_start(
                            xld[:], x_d[row0 : row0 + 128, kc * 768 : (kc + 1) * 768])
                        xbf = xcast.tile([128, 768], MM_DT, name="xbf", tag="xbf")
                        nc.vector.tensor_copy(xbf[:], xld[:])
                        for pi in range(6):
                            pt = kc * 6 + pi
                            pst = psum_t.tile([128, 128], MM_DT, name="pst", tag="pst")
                            nc.tensor.transpose(
                                pst[:], xbf[:, pi * 128 : (pi + 1) * 128], ident[:])
                            nc.vector.tensor_copy(
                                xT[:, pt * NB + sub * 128 : pt * NB + sub * 128 + 128],
                                pst[:],
                            )

                # input tiles of layer 1: block k, K-tile ki -> (col, ksize)
                in_tiles = [
                    [((k * 6 + ki) * NB, 128) for ki in range(6)] for k in range(4)
                ]
                h = xT

                for li, (p_dim, q_dim, chunk, s_dim) in enumerate(LAYER_CFG, 1):
                    QPAD = 4 * chunk
                    ntl = QPAD // 128  # plane tiles
                    kts = ktiles(p_dim)
                    nkt = len(kts)
                    # --- mm1: blocks -> plane-pure PSUM tiles -> planes SBUF
                    planes = acts.tile([128, 4 * ntl * NB], MM_DT, name=f"planes{li}", tag=f"planes{li}")
                    for l in range(4):
                        for t in range(ntl):
                            ps = psum_mm.tile([128, NB], F32, name="ps_mm", tag="ps_mm")
                            for (pc0, pc1, k, qc0) in pieces_for_tile(t, chunk):
                                for ki, ((k0, ksz), (hcol, _)) in enumerate(
                                    zip(kts, in_tiles[k])
                                ):
                                    wcol = (k * nkt + ki) * QPAD + l * chunk + qc0
                                    nc.tensor.matmul(
                                        ps[pc0:pc1, :],
                                        w1sb[li][:ksz, wcol : wcol + (pc1 - pc0)],
                                        h[:ksz, hcol : hcol + NB],
                                        start=(ki == 0),
                                        stop=(ki == nkt - 1),
                                        tile_position=(0, pc0),
                                    )
                            nc.vector.tensor_copy(
                                planes[:, (l * ntl + t) * NB : (l * ntl + t + 1) * NB],
                                ps[:],
                            )

                    if li < 4:
                        # --- mm2: planes -> next-layer blocks (relu+bias on evict)
                        mts = ktiles(s_dim)
                        nmt = len(mts)
                        hn = acts.tile([128, 4 * nmt * NB], MM_DT, name=f"h{li + 1}", tag=f"h{li + 1}")
                        for l in range(4):
                            for mi, (m0, msz) in enumerate(mts):
                                ps = psum_mm.tile([128, NB], F32, name="ps_mm", tag="ps_mm")
                                for rt in range(ntl):
                                    wcol = (l * ntl + rt) * s_dim + m0
                                    nc.tensor.matmul(
                                        ps[:msz, :],
                                        w2sb[li][:, wcol : wcol + msz],
                                        planes[:, (l * ntl + rt) * NB : (l * ntl + rt + 1) * NB],
                                        start=(rt == 0),
                                        stop=(rt == ntl - 1),
                                    )
                                hcol = (l * nmt + mi) * NB
                                nc.scalar.activation(
                                    hn[:msz, hcol : hcol + NB],
                                    ps[:msz, :],
                                    mybir.ActivationFunctionType.Relu,
                                    bias=biassb[li][:msz, l * nmt + mi : l * nmt + mi + 1],
                                )
                        in_tiles = [
                            [((l * nmt + mi) * NB, msz) for mi, (m0, msz) in enumerate(mts)]
                            for l in range(4)
                        ]
                        h = hn
                    else:
                        # --- mm2-L4 swapped: activations stationary -> [b, 12]
                        for sub in range(nsub):
                            ps4 = psum_s.tile([128, 12], F32, name="ps4", tag="ps4")
                            for l in range(4):
                                nc.tensor.matmul(
                                    ps4[:, :],
                                    planes[:, l * NB + sub * 128 : l * NB + sub * 128 + 128],
                                    w2sb[4][:, l * 12 : (l + 1) * 12],
                                    start=(l == 0),
                                    stop=False,
                                )
                            nc.tensor.matmul(
                                ps4[:, :],
                                ones_row[:1, :],
                                biassb[4][:1, :],
                                start=False,
                                stop=True,
                            )
                            # log_softmax over cols 0..9 (f32)
                            sm = smp.tile([128, 10], F32, name="sm", tag="sm")
                            nc.vector.tensor_copy(sm[:], ps4[:, 0:10])
                            mx = smp.tile([128, 1], F32, name="mx", tag="mx")
                            nc.vector.reduce_max(mx[:], sm[:], axis=mybir.AxisListType.X)
                            nmx = smp.tile([128, 1], F32, name="nmx", tag="nmx")
                            nc.scalar.mul(nmx[:], mx[:], -1.0)
                            ex = smp.tile([128, 10], F32, name="ex", tag="ex")
                            nc.scalar.activation(
                                ex[:], sm[:], mybir.ActivationFunctionType.Exp,
                                bias=nmx[:])
                            sme = smp.tile([128, 1], F32, name="sme", tag="sme")
                            nc.vector.reduce_sum(sme[:], ex[:], axis=mybir.AxisListType.X)
                            lse = smp.tile([128, 1], F32, name="lse", tag="lse")
                            nc.scalar.activation(
                                lse[:], sme[:], mybir.ActivationFunctionType.Ln)
                            ofs = smp.tile([128, 1], F32, name="ofs", tag="ofs")
                            nc.vector.tensor_sub(ofs[:], nmx[:], lse[:])
                            out_t = smp.tile([128, 10], F32, name="out_t", tag="out_t")
                            nc.scalar.activation(
                                out_t[:], sm[:],
                                mybir.ActivationFunctionType.Identity,
                                bias=ofs[:])
                            row0 = bt * NB + sub * 128
                            nc.sync.dma_start(y_d[row0 : row0 + 128, :], out_t[:])
    nc.compile()
    return nc


def kernel(**inputs):
    x = np.ascontiguousarray(np.asarray(inputs["x"], np.float32))
    arrs = prepare_weights(inputs)
    nc = build_nc()
    in_maps = []
    for c in range(N_CORES):
        m = dict(arrs)
        m["x"] = x[c * B_CORE : (c + 1) * B_CORE]
        in_maps.append(m)
    res = run_bass_kernel_spmd(nc, in_maps, list(range(N_CORES))).results
    return np.concatenate([r["y"] for r in res], axis=0)


# revision 18
# speedup vs baseline: 204.3548x; 204.3548x over previous
"""Trainium2 Bass kernel for nn_CIFAR10_Monarch_MLP2 (4-layer Monarch MLP + log_softmax).

Strategy
--------
Data-parallel over 8 NeuronCores: each core computes 2048 rows of the
16384-row batch with replicated weights; outputs are concatenated on host.

Per core, activations are kept feature-major ([feature partitions, batch
free]) so the block-diagonal matmuls chain without transposes.  The monarch
permutation (flat index k*q+qq -> plane l=(f%4), row r=f//4) is folded into a
host-side re-arrangement of the weights:

 * w1 rows of block k are regrouped by destination plane l, each group padded
   to a fixed `chunk` (multiple of 32), so mm1's PSUM output tiles are
   "plane-pure": every 128-partition PSUM tile belongs to a single plane and
   is written by 1-4 column-offset matmuls (32-aligned tile_position), one per
   contributing source block.
 * w2 columns are permuted to match the resulting plane-row order (pad rows
   get zero columns), so no data movement is needed for the permutation.

x arrives batch-major in HBM; it is DMA'd contiguously, cast to the matmul
dtype, and transposed on the PE (128x128 identity transposes) into
feature-major tiles.  The final layer's second matmul swaps stationary and
moving operands (activations become lhsT) so its output lands batch-major,
where log_softmax is a cheap free-dim reduction, and results DMA straight out.
"""

import numpy as np
import ml_dtypes

import concourse.bass as bass
from concourse import bacc
import concourse.mybir as mybir
import concourse.tile as tile
from concourse.bass_utils import run_bass_kernel_spmd
from concourse.masks import make_identity

F32 = mybir.dt.float32

# matmul operand dtype knob: mybir.dt.bfloat16 | float32r | float32
MM_DT = mybir.dt.bfloat16

N_CORES = 8
BATCH = 16384
B_CORE = BATCH // N_CORES  # 2048
NB = 256  # batch-tile free size (PSUM bank = 2KB -> 512 fp32 max; 256 = half)

# (p_dim, q_dim, chunk, s_dim) per layer; QPAD = RPAD = 4*chunk
LAYER_CFG = [
    (768, 750, 192, 750),
    (750, 250, 64, 250),
    (250, 25, 32, 25),
    (25, 3, 32, 3),
]


def _np_mmdt():
    return {
        mybir.dt.bfloat16: ml_dtypes.bfloat16,
        mybir.dt.float32r: np.float32,
        mybir.dt.float32: np.float32,
    }[MM_DT]


def arrange_layer(w1, w2, q_dim, chunk):
    """w1:(4,q,p), w2:(4,s,r=q) -> w1t:[4,p,QPAD] (mm1 lhsT), w2t:[4,QPAD,s]
    (mm2 lhsT), with the monarch permutation folded in (see module doc)."""
    nb, _, p_dim = w1.shape
    s_dim = w2.shape[1]
    QPAD = 4 * chunk
    w1t = np.zeros((nb, p_dim, QPAD), np.float32)
    w2t = np.zeros((nb, QPAD, s_dim), np.float32)
    for k in range(nb):
        for l in range(nb):
            qs = [q for q in range(q_dim) if (k * q_dim + q) % 4 == l]
            w1t[k, :, l * chunk : l * chunk + len(qs)] = w1[k, qs, :].T
            rs = [(k * q_dim + q) // 4 for q in qs]
            w2t[l, k * chunk : k * chunk + len(qs), :] = w2[l, :, rs]
    return w1t, w2t


def evict_frags(k, m, chunk):
    """Fragments to scatter mm1's natural PSUM M-tile m of block k (padded
    rows [128m, 128m+128)) into the plane layout.

    Returns [(src_part0, size, plane_l, plane_tile, dst_part_base), ...].
    Fragment boundaries lie on the src 128-grid, dst 128-grid and l-chunk
    grid; shifted fragments are split to the DVE-legal 64 (or 32) grain.
    """
    grain = 64 if chunk % 64 == 0 else 32
    frags = []
    g = 128 * m
    end = 128 * (m + 1)
    while g < end:
        l = g // chunk
        dst = k * chunk + (g - l * chunk)  # global row within plane l
        # next boundary: chunk end, src tile end, dst tile end
        nb_ = min(end, (l + 1) * chunk, g + (128 - dst % 128))
        size = nb_ - g
        src_b = g - 128 * m
        dst_b = dst % 128
        if src_b % 128 == dst_b:
            frags.append((src_b, size, l, dst // 128, dst_b))
            g = nb_
        else:
            # shifted: emit at grain granularity (64: halves; 32: quadrants)
            step = min(grain, size)
            frags.append((src_b, step, l, dst // 128, dst_b))
            g += step
    return frags


def ktiles(p_dim):
    """[(row0, size), ...] 128-partition contraction tiles covering p_dim."""
    return [(r, min(128, p_dim - r)) for r in range(0, p_dim, 128)]


def prepare_weights(inputs):
    """Host-side arrangement of all weights/biases into DRAM-parameter arrays."""
    npdt = _np_mmdt()
    arrs = {}
    for li, (p_dim, q_dim, chunk, s_dim) in enumerate(LAYER_CFG, 1):
        w1 = np.asarray(inputs[f"w1_{li}"], np.float32)
        w2 = np.asarray(inputs[f"w2_{li}"], np.float32)
        w1t, w2t = arrange_layer(w1, w2, q_dim, chunk)
        arrs[f"w1t_{li}"] = w1t.astype(npdt)
        if li <= 2:
            arrs[f"w2t_{li}"] = w2t.astype(npdt)
            bias = np.asarray(inputs[f"b{li}"], np.float32)  # [4*s_dim], f'=l*s+s
            # bias columns per (plane l, s-tile mt): [128, ncols]
            mts = ktiles(s_dim)
            cols = np.zeros((128, 4 * len(mts)), np.float32)
            for l in range(4):
                for mi, (m0, msz) in enumerate(mts):
                    cols[:msz, l * len(mts) + mi] = bias[l * s_dim + m0 : l * s_dim + m0 + msz]
            arrs[f"bias_{li}"] = cols
        elif li == 3:
            # mixed per-block mm2 weights: contract g3[k] (padded row l*32+i)
            # directly; output rows on the padded f''=l*32+s grid.
            w2mix = np.zeros((4, 128, 128), np.float32)
            for k in range(4):
                for l in range(4):
                    qs = [q for q in range(q_dim) if (k * q_dim + q) % 4 == l]
                    for i, q in enumerate(qs):
                        r = (k * q_dim + q) // 4
                        w2mix[k, l * 32 + i, l * 32 : l * 32 + s_dim] = w2[l, :, r]
            arrs["w2mix_3"] = w2mix.astype(npdt)
            b3 = np.asarray(inputs["b3"], np.float32)
            b3c = np.zeros((128, 1), np.float32)
            for l in range(4):
                b3c[l * 32 : l * 32 + s_dim, 0] = b3[l * s_dim : (l + 1) * s_dim]
            arrs["bias_3"] = b3c
        else:
            # L4: per-block mixed weights for the operand-swapped mm2:
            # out[b, 3l+s] += sum_row g4[k][row, b] * w2bigk[k][row, 3l+s]
            w2bigk = np.zeros((4, 128, 12), np.float32)
            for k in range(4):
                for l in range(4):
                    qs = [q for q in range(q_dim) if (k * q_dim + q) % 4 == l]
                    for i, q in enumerate(qs):
                        r = (k * q_dim + q) // 4
                        w2bigk[k, l * 32 + i, 3 * l : 3 * l + 3] = w2[l, :, r]
            arrs["w2bigk_4"] = w2bigk.astype(npdt)
            b4 = np.asarray(inputs["b4"], np.float32)
            b4r = np.zeros((1, 12), np.float32)
            b4r[0, :10] = b4
            arrs["b4r"] = b4r.astype(npdt)
    return arrs


def build_nc(b_core=B_CORE, repeat=1):
    """Build the single-core Bass program (SPMD: same program, per-core x).
    repeat>1 re-runs the whole batch pipeline (for timing-by-differencing)."""
    nc = bacc.Bacc(None, target_bir_lowering=False)
    x_d = nc.declare_dram_parameter("x", [b_core, 3072], F32, isOutput=False)
    y_d = nc.declare_dram_parameter("y", [b_core, 10], F32, isOutput=True)

    wd = {}
    for li, (p_dim, q_dim, chunk, s_dim) in enumerate(LAYER_CFG, 1):
        QPAD = 4 * chunk
        wd[f"w1t_{li}"] = nc.declare_dram_parameter(
            f"w1t_{li}", [4, p_dim, QPAD], MM_DT, isOutput=False)
        if li <= 2:
            wd[f"w2t_{li}"] = nc.declare_dram_parameter(
                f"w2t_{li}", [4, QPAD, s_dim], MM_DT, isOutput=False)
            nmt = len(ktiles(s_dim))
            wd[f"bias_{li}"] = nc.declare_dram_parameter(
                f"bias_{li}", [128, 4 * nmt], F32, isOutput=False)
        elif li == 3:
            wd["w2mix_3"] = nc.declare_dram_parameter(
                "w2mix_3", [4, 128, 128], MM_DT, isOutput=False)
            wd["bias_3"] = nc.declare_dram_parameter("bias_3", [128, 1], F32, isOutput=False)
        else:
            wd["w2bigk_4"] = nc.declare_dram_parameter(
                "w2bigk_4", [4, 128, 12], MM_DT, isOutput=False)
            wd["b4r"] = nc.declare_dram_parameter("b4r", [1, 12], MM_DT, isOutput=False)

    n_bt = b_core // NB

    with tile.TileContext(nc) as tc:
        with (
            tc.tile_pool(name="const", bufs=1) as const,
            tc.tile_pool(name="xload", bufs=3) as xload,
            tc.tile_pool(name="xcast", bufs=3) as xcast,
            tc.tile_pool(name="xT", bufs=2) as xTp,
            tc.tile_pool(name="acts", bufs=1) as acts,
            tc.tile_pool(name="psum_mm", bufs=4, space="PSUM") as psum_mm,
            tc.tile_pool(name="psum_s", bufs=2, space="PSUM") as psum_s,
            tc.tile_pool(name="sm", bufs=2) as smp,
        ):
            # ---- resident constants ----
            ones_row = const.tile([1, 128], MM_DT, name="ones_row", tag="ones_row")
            nc.any.memset(ones_row[:], 1.0)

            w1sb, w2sb, biassb = {}, {}, {}
            for li, (p_dim, q_dim, chunk, s_dim) in enumerate(LAYER_CFG, 1):
                QPAD = 4 * chunk
                kts = ktiles(p_dim)
                w1sb[li] = const.tile([128, len(kts) * 4 * QPAD], MM_DT, name=f"w1sb{li}", tag=f"w1sb{li}")
                for k in range(4):
                    p0 = 32 * k if li == 4 else 0  # L4: block k at partitions 32k
                    for ki, (k0, ksz) in enumerate(kts):
                        col = (k * len(kts) + ki) * QPAD
                        nc.gpsimd.dma_start(
                            w1sb[li][p0 : p0 + ksz, col : col + QPAD],
                            wd[f"w1t_{li}"][k, k0 : k0 + ksz, :],
                        )
                if li <= 2:
                    nrt = QPAD // 128
                    w2sb[li] = const.tile([128, 4 * nrt * s_dim], MM_DT, name=f"w2sb{li}", tag=f"w2sb{li}")
                    for l in range(4):
                        for rt in range(nrt):
                            col = (l * nrt + rt) * s_dim
                            nc.gpsimd.dma_start(
                                w2sb[li][:, col : col + s_dim],
                                wd[f"w2t_{li}"][l, 128 * rt : 128 * (rt + 1), :],
                            )
                    nmt = len(ktiles(s_dim))
                    biassb[li] = const.tile([128, 4 * nmt], F32, name=f"biassb{li}", tag=f"biassb{li}")
                    nc.gpsimd.dma_start(biassb[li][:], wd[f"bias_{li}"][:, :])
                elif li == 3:
                    w2sb[3] = const.tile([128, 4 * 128], MM_DT, name="w2sb3", tag="w2sb3")
                    for k in range(4):
                        nc.gpsimd.dma_start(
                            w2sb[3][:, k * 128 : (k + 1) * 128], wd["w2mix_3"][k, :, :])
                    biassb[3] = const.tile([128, 1], F32, name="biassb3", tag="biassb3")
                    nc.gpsimd.dma_start(biassb[3][:], wd["bias_3"][:, :])
                else:
                    w2sb[4] = const.tile([128, 4 * 12], MM_DT, name="w2sb4", tag="w2sb4")
                    for k in range(4):
                        nc.gpsimd.dma_start(
                            w2sb[4][:, k * 12 : (k + 1) * 12], wd["w2bigk_4"][k, :, :])
                    biassb[4] = const.tile([1, 12], MM_DT, name="b4rsb", tag="b4rsb")
                    nc.gpsimd.dma_start(biassb[4][:], wd["b4r"][:, :])

            # ---- batch-tile pipeline ----
            last_tr = None  # previous tile's final xbar transpose
            for bt in [t for _ in range(repeat) for t in range(n_bt)]:
                nsub = NB // 128
                # x: load batch-major (SWDGE), cast to bf16, xbar-transpose
                # (SBUF->SBUF DMA transpose) into feature-major xT.
                # xT layout: [128, nsub*3072], sub-major: col = sub*3072 + pt*128 + b
                xT = xTp.tile([128, nsub * 3072], MM_DT, name="xT", tag="xT")
                xbfs = []
                for sub in range(nsub):
                    row0 = bt * NB + sub * 128
                    xld = xload.tile([128, 3072], F32, name="xld", tag="xld")
                    eng = nc.sync if sub % 2 == 0 else nc.scalar
                    ldi = eng.dma_start(xld[:], x_d[row0 : row0 + 128, :])
                    if last_tr is not None:
                        # keep this tile's loads behind the previous tile's
                        # xbar transposes (xbar-mode switch serializes DMA)
                        tile.add_dep_helper(ldi.ins, last_tr.ins, sync=True,
                                            reason="xld after prev transpose")
                    xbf = xcast.tile([128, 3072], MM_DT, name="xbf", tag="xbf")
                    nc.vector.tensor_copy(xbf[:], xld[:])
                    xbfs.append(xbf)
                for sub in range(nsub):
                    xTv = xT[:, sub * 3072 : (sub + 1) * 3072].rearrange(
                        "p (g b) -> p g b", b=128)
                    last_tr = nc.sync.dma_start(xTv, xbfs[sub][:, :], transpose=True)

                # layer-1 rhs: block k, K-tile ki -> [128, (sub, b)] 2-dim-free AP
                xTr = xT.rearrange("p (s c) -> p s c", s=nsub)
                l1_rhs = [
                    [xTr[:, :, (k * 6 + ki) * 128 : (k * 6 + ki + 1) * 128]
                     for ki in range(6)]
                    for k in range(4)
                ]
                h = None

                for li, (p_dim, q_dim, chunk, s_dim) in enumerate(LAYER_CFG[:2], 1):
                    QPAD = 4 * chunk
                    ntl = QPAD // 128  # plane tiles
                    kts = ktiles(p_dim)
                    nkt = len(kts)
                    # --- mm1: natural block M-tiles (M=128, no col splits);
                    # evictions scatter to plane layout via (possibly
                    # partition-shifted) DVE fragment copies.  Legal shifts:
                    # any size at shift 0; 64-sized between halves; 32-sized
                    # between quadrants (HW-verified quadrant routing).
                    planes = acts.tile([128, 4 * ntl * NB], MM_DT, name=f"planes{li}", tag=f"planes{li}")
                    for k in range(4):
                        for m in range(ntl):
                            ps = psum_mm.tile([128, NB], F32, name="ps_mm", tag="ps_mm")
                            for ki, (k0, ksz) in enumerate(kts):
                                if li == 1:
                                    rhs = l1_rhs[k][ki]
                                else:
                                    hcol = in_tiles[k][ki][0]
                                    rhs = h[:ksz, hcol : hcol + NB]
                                wcol = (k * nkt + ki) * QPAD + 128 * m
                                nc.tensor.matmul(
                                    ps[:, :],
                                    w1sb[li][:ksz, wcol : wcol + 128],
                                    rhs,
                                    start=(ki == 0),
                                    stop=(ki == nkt - 1),
                                )
                            for (s0, sz, l, jt, db) in evict_frags(k, m, chunk):
                                pcol = (l * ntl + jt) * NB
                                # shifted copies need the DVE output crossbar;
                                # in-lane ones can go to whichever engine is idle
                                eng = nc.vector if s0 % 128 != db else nc.any
                                eng.tensor_copy(
                                    planes[db : db + sz, pcol : pcol + NB],
                                    ps[s0 : s0 + sz, :],
                                )

                    # --- mm2: planes -> next-layer blocks (relu+bias on evict)
                    mts = ktiles(s_dim)
                    nmt = len(mts)
                    hn = acts.tile([128, 4 * nmt * NB], MM_DT, name=f"h{li + 1}", tag=f"h{li + 1}")
                    for l in range(4):
                        for mi, (m0, msz) in enumerate(mts):
                            ps = psum_mm.tile([128, NB], F32, name="ps_mm", tag="ps_mm")
                            for rt in range(ntl):
                                wcol = (l * ntl + rt) * s_dim + m0
                                nc.tensor.matmul(
                                    ps[:msz, :],
                                    w2sb[li][:, wcol : wcol + msz],
                                    planes[:, (l * ntl + rt) * NB : (l * ntl + rt + 1) * NB],
                                    start=(rt == 0),
                                    stop=(rt == ntl - 1),
                                )
                            hcol = (l * nmt + mi) * NB
                            nc.scalar.activation(
                                hn[:msz, hcol : hcol + NB],
                                ps[:msz, :],
                                mybir.ActivationFunctionType.Relu,
                                bias=biassb[li][:msz, l * nmt + mi : l * nmt + mi + 1],
                            )
                    in_tiles = [
                        [((l * nmt + mi) * NB, msz) for mi, (m0, msz) in enumerate(mts)]
                        for l in range(4)
                    ]
                    h = hn

                # ---- L3: natural mm1 -> g3 blocks; mm2 with per-block mixed
                # weights contracts g3 directly (no plane materialization).
                kts3 = ktiles(250)
                g3 = acts.tile([128, 4 * NB], MM_DT, name="g3", tag="g3")
                for k in range(4):
                    ps = psum_mm.tile([128, NB], F32, name="ps_mm", tag="ps_mm")
                    for ki, (k0, ksz) in enumerate(kts3):
                        hcol = in_tiles[k][ki][0]
                        nc.tensor.matmul(
                            ps[:, :],
                            w1sb[3][:ksz, (k * 2 + ki) * 128 : (k * 2 + ki + 1) * 128],
                            h[:ksz, hcol : hcol + NB],
                            start=(ki == 0),
                            stop=(ki == 1),
                        )
                    nc.any.tensor_copy(g3[:, k * NB : (k + 1) * NB], ps[:, :])
                h4 = acts.tile([128, NB], MM_DT, name="h4", tag="h4")
                ps3 = psum_mm.tile([128, NB], F32, name="ps_mm", tag="ps_mm")
                for k in range(4):
                    nc.tensor.matmul(
                        ps3[:, :],
                        w2sb[3][:, k * 128 : (k + 1) * 128],
                        g3[:, k * NB : (k + 1) * NB],
                        start=(k == 0),
                        stop=(k == 3),
                    )
                nc.scalar.activation(
                    h4[:, :], ps3[:, :], mybir.ActivationFunctionType.Relu,
                    bias=biassb[3][:, 0:1])

                # ---- L4: K=25 mm1 per block at partition base 32k -> g4;
                # operand-swapped mm2 accumulates [b, 12] batch-major.
                g4 = acts.tile([128, 4 * NB], MM_DT, name="g4", tag="g4")
                for k in range(4):
                    ps = psum_mm.tile([128, NB], F32, name="ps_mm", tag="ps_mm")
                    nc.tensor.matmul(
                        ps[:, :],
                        w1sb[4][32 * k : 32 * k + 25, k * 128 : (k + 1) * 128],
                        h4[32 * k : 32 * k + 25, :],
                        start=True,
                        stop=True,
                        tile_position=(32 * k, 0),
                    )
                    nc.any.tensor_copy(g4[:, k * NB : (k + 1) * NB], ps[:, :])
                for sub in range(nsub):
                    ps4 = psum_s.tile([128, 12], F32, name="ps4", tag="ps4")
                    for k in range(4):
                        nc.tensor.matmul(
                            ps4[:, :],
                            g4[:, k * NB + sub * 128 : k * NB + sub * 128 + 128],
                            w2sb[4][:, k * 12 : (k + 1) * 12],
                            start=(k == 0),
                            stop=False,
                        )
                    nc.tensor.matmul(
                        ps4[:, :],
                        ones_row[:1, :],
                        biassb[4][:1, :],
                        start=False,
                        stop=True,
                    )
                    # log_softmax over cols 0..9 (f32)
                    sm = smp.tile([128, 10], F32, name="sm", tag="sm")
                    nc.vector.tensor_copy(sm[:], ps4[:, 0:10])
                    mx = smp.tile([128, 1], F32, name="mx", tag="mx")
                    nc.vector.reduce_max(mx[:], sm[:], axis=mybir.AxisListType.X)
                    nmx = smp.tile([128, 1], F32, name="nmx", tag="nmx")
                    nc.scalar.mul(nmx[:], mx[:], -1.0)
                    ex = smp.tile([128, 10], F32, name="ex", tag="ex")
                    nc.scalar.activation(
                        ex[:], sm[:], mybir.ActivationFunctionType.Exp, bias=nmx[:])
                    sme = smp.tile([128, 1], F32, name="sme", tag="sme")
                    nc.vector.reduce_sum(sme[:], ex[:], axis=mybir.AxisListType.X)
                    lse = smp.tile([128, 1], F32, name="lse", tag="lse")
                    nc.scalar.activation(
                        lse[:], sme[:], mybir.ActivationFunctionType.Ln)
                    ofs = smp.tile([128, 1], F32, name="ofs", tag="ofs")
                    nc.vector.tensor_sub(ofs[:], nmx[:], lse[:])
                    out_t = smp.tile([128, 10], F32, name="out_t", tag="out_t")
                    nc.scalar.activation(
                        out_t[:], sm[:], mybir.ActivationFunctionType.Identity,
                        bias=ofs[:])
                    row0 = bt * NB + sub * 128
                    nc.sync.dma_start(y_d[row0 : row0 + 128, :], out_t[:])
    nc.compile()
    return nc


def kernel(**inputs):
    x = np.ascontiguousarray(np.asarray(inputs["x"], np.float32))
    arrs = prepare_weights(inputs)
    nc = build_nc()
    in_maps = []
    for c in range(N_CORES):
        m = dict(arrs)
        m["x"] = x[c * B_CORE : (c + 1) * B_CORE]
        in_maps.append(m)
    res = run_bass_kernel_spmd(nc, in_maps, list(range(N_CORES))).results
    return np.concatenate([r["y"] for r in res], axis=0)
